# revision 3
# baseline (speedup 1.0000x reference)
"""Trainium2 Bass kernel for nn_ArithmeticExperts (reciprocal_table).

Reference math per element:
    sign = sign(x); xa = |x|
    exp  = floor(log2(xa)) + 1 ; temp = xa * 2^-exp  (mantissa in [0.5, 1))
    idx  = (temp - 0.5) * 256
    y0   = softmax(-|arange(256) - idx| * 1000) @ table   # sharp softmax
    y    = y0*(2 - temp*y0); y = y*(2 - temp*y)           # 2 Newton steps
    out  = y * 2^-exp * sign

Key observation: the reference's 8-bit table seed + two Newton steps converge
to 1/x at f32 roundoff (seed rel err ~2e-3 -> (2e-3)^4 after two steps, far
below f32 eps), so its output IS 1/x up to a few ulp.  A single DVE
InstReciprocal (IEEE-exact 1/x on TRN2, bitwise-verified by the interpreter
suite and measured 0.0 rel err vs np.reciprocal on these inputs across
repeated device runs) replaces the whole 12-op table+Newton pipeline.

What remains is DMA choreography; per core (65536 elems = [128 part, 512]):
  - two SP-issued input HWDGE DMAs (260/252 cols).  Each has its OWN
    semaphore: DMA completions across queues are not ordered, so a shared
    counting semaphore would race.
  - two DVE reciprocals, each with its input-wait ATTACHED to the
    instruction (waits ride the engine-stage for free; standalone
    EventSemaphore ops would cost ~70ns SEQ each and serialize dispatch).
  - two output HWDGE DMAs: first (cols 0-259) from ACT as soon as recip0's
    sem fires, second (cols 260-511) from SP after recip1; the split lets
    the first output's descriptor-gen overlap recip1 + keeps the final
    transfer small.  walrus requires every DGE DMA to carry >=1 sem update,
    so both increment s_od (nothing waits on it; outputs are complete when
    the NEFF drains -- verified bit-exact over repeated executions).
  - Bass's per-engine register preamble (5 RegisterMoves/engine, ~300ns
    before SP's first instruction), the const-AP memsets and the all-engine
    startup barrier are patched out during Bass construction: this kernel
    uses none of them, and they sit directly on the critical path.

Timeline (cost model): in-issue 0/650 -> DGE 1300 -> xfers -> +900ns DMA
sem -> recip0 ~2640, recip1 ~3540 -> out HWDGEs (ACT then SP) -> final
transfer ends ~5270 -> +900ns completion-sem propagation = ~6165ns, vs
11338ns for the table+Newton baseline.

Dead ends (tried, kept out): SWDGE prepare_only + trigger_dma would cut the
tail to ~4.4us, but this container's walrus rejects InstTriggerDma ("ISA
wrong length", with or without signals/register count); kv_writeback
(normal mode) compiles and prices at 9 descriptors (~51ns transfer) but its
Pool ucode crashes at execution; Pool-issued plain DMAs gain <10ns in the
model and add ucode risk; ACT-engine Reciprocal is rejected by bass for
accuracy and loses to DVE anyway (222-cycle SBUF access); gather/scatter
price at full per-row descriptor cost.

Pure data parallel: 8 cores x 65536 contiguous elements, no collectives.
"""

import sys

if "/opt/trn_rl_repo" not in sys.path:
    sys.path.insert(0, "/opt/trn_rl_repo")

import numpy as np

N = 524288
N_CORES = 8
SHARD = N // N_CORES          # 65536
P = 128
F = SHARD // P                # 512 elements per partition


def _build_bass(in_tiles=((260, "sync"), (252, "sync")),
                out_tiles=((260, "scalar"), (252, "sync")),
                recip_tiles=None, strip_preamble=True,
                final_wait=False, end_mode="drains", clears=True,
                attach_waits=True):
    """in_tiles: (cols, engine) per input DMA, each with its own semaphore.
    out_tiles: (cols, engine) per output DMA; recip granularity =
    recip_tiles or in_tile widths (must nest within in_tile boundaries)."""
    import contextlib

    import concourse.bass as bass
    import concourse.mybir as mybir

    f32 = mybir.dt.float32
    in_tiles = [tuple(t) for t in in_tiles]
    out_tiles = [tuple(t) for t in out_tiles]
    assert sum(w for w, _ in in_tiles) == F
    assert sum(w for w, _ in out_tiles) == F
    # only these engines have emit paths below; anything else would be
    # silently dropped (sim then reports a bogus-fast, output-less kernel)
    assert all(e in ("sync", "scalar", "gpsimd") for _, e in in_tiles + out_tiles)
    if recip_tiles is None:
        recip_tiles = [w for w, _ in in_tiles]
    recip_tiles = list(recip_tiles)
    assert sum(recip_tiles) == F

    # Bass.__init__ emits, per engine, a 5-RegisterMove preamble plus 4
    # const-AP memsets and an all-engine startup barrier.  None are used by
    # this kernel and the SP preamble delays the first input DMA by ~300ns,
    # so patch them out for the duration of construction.
    patches = []

    def patch(cls, name, val):
        patches.append((cls, name, name in vars(cls), getattr(cls, name, None)))
        setattr(cls, name, val)

    patch(bass.Bass, "all_engine_barrier", lambda self, **kw: None)
    seen = set()
    for cls_name in dir(bass):
        cls = getattr(bass, cls_name)
        if not isinstance(cls, type) or cls in seen or not cls_name.startswith("Bass"):
            continue
        seen.add(cls)
        if hasattr(cls, "memset"):
            patch(cls, "memset", lambda self, ap, c: None)
        if strip_preamble and hasattr(cls, "preamble"):
            patch(cls, "preamble", lambda self: None)
    try:
        nc = bass.Bass(trn_type="TRN2")
    finally:
        for cls, name, had, orig in patches:
            if had:
                setattr(cls, name, orig)
            else:
                try:
                    delattr(cls, name)
                except AttributeError:
                    pass

    x_d = nc.dram_tensor("x", [P, F], f32, kind="ExternalInput")
    o_d = nc.dram_tensor("out", [P, F], f32, kind="ExternalOutput")

    in_off = [sum(w for w, _ in in_tiles[:i]) for i in range(len(in_tiles))]
    out_off = [sum(w for w, _ in out_tiles[:i]) for i in range(len(out_tiles))]
    r_off = [sum(recip_tiles[:i]) for i in range(len(recip_tiles))]
    n_in = len(in_tiles)
    n_out = len(out_tiles)
    n_r = len(recip_tiles)

    def in_idx_for(a, b):
        """index of the single in-tile containing columns [a, b)."""
        for i in range(n_in):
            if in_off[i] <= a and b <= in_off[i] + in_tiles[i][0]:
                return i
        raise AssertionError(f"recip [{a},{b}) spans in-tile boundary")

    def r_count_for(a, b):
        """number of leading recips that must complete before reading
        columns [a, b) of the result."""
        k = 0
        for i in range(n_r):
            if r_off[i] < b and r_off[i] + recip_tiles[i] > a:
                k = i + 1
        return k

    with contextlib.ExitStack() as st:
        ent = st.enter_context
        xt = ent(nc.sbuf_tensor([P, F], f32))
        ot = ent(nc.sbuf_tensor([P, F], f32))

        s_ins = [ent(nc.semaphore(name=f"s_in{i}")) for i in range(n_in)]
        s_dve = ent(nc.semaphore(name="s_dve"))
        s_od = ent(nc.semaphore(name="s_od"))

        eng_of = {"sync": nc.sync, "scalar": nc.scalar, "vector": nc.vector,
                  "gpsimd": nc.gpsimd}

        def emit_in(i):
            w, e = in_tiles[i]
            c0 = in_off[i]
            eng_of[e].dma_start(
                xt[:, c0:c0 + w], x_d[:, c0:c0 + w]).then_inc(s_ins[i], 16)

        def emit_out(j):
            w, e = out_tiles[j]
            c0 = out_off[j]
            if not attach_waits:
                eng_of[e].wait_ge(s_dve, r_count_for(c0, c0 + w))
            ins = eng_of[e].dma_start(o_d[:, c0:c0 + w], ot[:, c0:c0 + w])
            if attach_waits:
                ins._wait_ge(s_dve, r_count_for(c0, c0 + w))
            # walrus requires every DGE DMA to carry >=1 sem update
            ins.then_inc(s_od, 16)

        # input DMAs go in the entry block, ahead of the branch into blk,
        # so SP's first instruction at t=0 is the first DMACopy
        for i in range(n_in):
            if in_tiles[i][1] == "sync":
                emit_in(i)

        blk = bass.BassBlock(nc, "blk")
        blk.__enter__()

        last_out_eng = out_tiles[-1][1]

        @blk.sync
        def _(sync):
            for j in range(n_out):
                if out_tiles[j][1] == "sync":
                    emit_out(j)
            if clears and last_out_eng == "sync":
                # last waiter of s_dve clears it so the loaded NEFF can be
                # re-executed (runs after this engine's last out-DMA wait,
                # hidden under the DMA flight)
                sync.wait_ge(s_dve, n_r)
                sync.sem_clear(s_dve)
            if final_wait:
                sync.wait_ge(s_od, 16 * n_out)
                sync.sem_clear(s_od)

        @blk.vector
        def _(vector):
            for j in range(n_r):
                w = recip_tiles[j]
                c0 = r_off[j]
                if not attach_waits:
                    vector.wait_ge(s_ins[in_idx_for(c0, c0 + w)], 16)
                ins = nc.vector.reciprocal(ot[:, c0:c0 + w], xt[:, c0:c0 + w])
                if attach_waits:
                    ins._wait_ge(s_ins[in_idx_for(c0, c0 + w)], 16)
                ins.then_inc(s_dve, 1)
            if clears:
                for i in range(n_in):
                    vector.wait_ge(s_ins[i], 16)
                    vector.sem_clear(s_ins[i])

        @blk.scalar
        def _(scalar):
            for i in range(n_in):
                if in_tiles[i][1] == "scalar":
                    emit_in(i)
            for j in range(n_out):
                if out_tiles[j][1] == "scalar":
                    emit_out(j)
            if clears and last_out_eng == "scalar":
                scalar.wait_ge(s_dve, n_r)
                scalar.sem_clear(s_dve)

        if any(e == "gpsimd" for _, e in in_tiles + out_tiles):
            @blk.gpsimd
            def _(gpsimd):
                for i in range(n_in):
                    if in_tiles[i][1] == "gpsimd":
                        emit_in(i)
                for j in range(n_out):
                    if out_tiles[j][1] == "gpsimd":
                        emit_out(j)
                if clears and last_out_eng == "gpsimd":
                    gpsimd.wait_ge(s_dve, n_r)
                    gpsimd.sem_clear(s_dve)

        for engine, last_body in blk.last_body.items():
            with nc.body(
                last_body, parent=nc.cur_bb, allow_existing_parent=True
            ):
                engine.br(blk.end_bb)
        nc.switch_bb(blk.end_bb)
        if end_mode == "drains":
            for eng_type, eng in nc.engines.items():
                d = mybir.InstDrain(
                    name=nc.get_next_instruction_name(),
                    ins=[], outs=[], bass_is_fusable=False,
                )
                d.engine = eng_type
                eng.add_instruction(d)

    return nc


_CACHED = {}

BEST_CONFIG = dict(
    in_tiles=((260, "sync"), (252, "sync")),
    out_tiles=((260, "scalar"), (252, "sync")),
)


def _freeze(v):
    if isinstance(v, (list, tuple)):
        return tuple(_freeze(x) for x in v)
    return v


def _get_nc(**kw):
    key = tuple(sorted((k, _freeze(v)) for k, v in kw.items()))
    if key not in _CACHED:
        _CACHED[key] = _build_bass(**dict(key))
    return _CACHED[key]


def kernel(x: np.ndarray, recip_table_val: np.ndarray = None, **_unused) -> np.ndarray:
    from concourse.bass_utils import run_bass_kernel_spmd

    x = np.ascontiguousarray(np.asarray(x, dtype=np.float32))
    assert x.shape == (N,), x.shape

    nc = _get_nc(**BEST_CONFIG)
    in_maps = [
        {"x": x[i * SHARD:(i + 1) * SHARD].reshape(P, F)} for i in range(N_CORES)
    ]
    res = run_bass_kernel_spmd(nc, in_maps, core_ids=list(range(N_CORES)))
    outs = [res.results[i]["out"].reshape(SHARD) for i in range(N_CORES)]
    return np.concatenate(outs).astype(np.float32)


if __name__ == "__main__":
    rng = np.random.default_rng(0)
    x = (rng.uniform(1.0, 1000.0, N) * np.where(rng.random(N) < 0.5, 1.0, -1.0)).astype(np.float32)
    y = kernel(x)
    print("ok", y[:4], 1.0 / x[:4])


# revision 7
# speedup vs baseline: 1.0026x; 1.0026x over previous
"""Trainium2 Bass kernel for nn_ArithmeticExperts (reciprocal_table).

Reference math per element:
    sign = sign(x); xa = |x|
    exp  = floor(log2(xa)) + 1 ; temp = xa * 2^-exp  (mantissa in [0.5, 1))
    idx  = (temp - 0.5) * 256
    y0   = softmax(-|arange(256) - idx| * 1000) @ table   # sharp softmax
    y    = y0*(2 - temp*y0); y = y*(2 - temp*y)           # 2 Newton steps
    out  = y * 2^-exp * sign

Key observation: the reference's 8-bit table seed + two Newton steps converge
to 1/x at f32 roundoff (seed rel err ~2e-3 -> (2e-3)^4 after two steps, far
below f32 eps), so its output IS 1/x up to a few ulp.  A single DVE
InstReciprocal (IEEE-exact 1/x on TRN2, bitwise-verified by the interpreter
suite and measured 0.0 rel err vs np.reciprocal on these inputs across
repeated device runs) replaces the whole 12-op table+Newton pipeline.

What remains is DMA choreography; per core (65536 elems = [128 part, 512]):
  - two input DMAs (244/268 cols): the first via SP's HWDGE (earliest
    possible transfer start, t=1300 in the model), the second via Pool's
    SWDGE (desc-gen on the otherwise-idle Pool engine, parallel to HWDGE;
    keeps the HWDGE free for the first output's early issue).  Each has
    its OWN semaphore: DMA completions across queues are not ordered, so
    a shared counting semaphore would race.
  - two DVE reciprocals, each with its input-wait ATTACHED to the
    instruction (waits ride the engine-stage for free; standalone
    EventSemaphore ops would cost ~70ns SEQ each and serialize dispatch).
  - two output HWDGE DMAs: first (cols 0-259) from ACT as soon as recip0's
    sem fires, second (cols 260-511) from SP after recip1; the split lets
    the first output's descriptor-gen overlap recip1 + keeps the final
    transfer small.  walrus requires every DGE DMA to carry >=1 sem update,
    so both increment s_od (nothing waits on it; outputs are complete when
    the NEFF drains -- verified bit-exact over repeated executions).
  - Bass's per-engine register preamble (5 RegisterMoves/engine, ~300ns
    before SP's first instruction), the const-AP memsets and the all-engine
    startup barrier are patched out during Bass construction: this kernel
    uses none of them, and they sit directly on the critical path.

Timeline (cost model): SP in-issue at t=0, Pool swdge-gen in parallel ->
xfers -> +900ns DMA sem -> recip0 ~2570, recip1 ~3110 -> out HWDGEs (ACT
at recip0-done, SP at recip1-done) -> final transfer ends ~5250 -> +900ns
completion-sem propagation = 6149ns, vs 11338ns for the table+Newton
baseline.  The two binding chains (first-tile recip -> ACT HWDGE-end, and
second-tile arrival -> recip -> SP HWDGE slot) are balanced to <90ns.

Dead ends (tried, kept out): SWDGE prepare_only + trigger_dma would cut the
tail to ~4.4us, but this container's walrus rejects InstTriggerDma ("ISA
wrong length", with or without signals/register count); kv_writeback
(normal mode) compiles and prices at 9 descriptors (~51ns transfer) but its
Pool ucode crashes at execution; Pool-issued plain DMAs gain <10ns in the
model and add ucode risk; ACT-engine Reciprocal is rejected by bass for
accuracy and loses to DVE anyway (222-cycle SBUF access); gather/scatter
price at full per-row descriptor cost.

Pure data parallel: 8 cores x 65536 contiguous elements, no collectives.
"""

import sys

if "/opt/trn_rl_repo" not in sys.path:
    sys.path.insert(0, "/opt/trn_rl_repo")

import numpy as np

N = 524288
N_CORES = 8
SHARD = N // N_CORES          # 65536
P = 128
F = SHARD // P                # 512 elements per partition


def _build_bass(in_tiles=((244, "sync"), (268, "gpsimd")),
                out_tiles=((244, "scalar"), (268, "sync")),
                recip_tiles=None, strip_preamble=True,
                final_wait=False, end_mode="drains", clears=True,
                attach_waits=True):
    """in_tiles: (cols, engine) per input DMA, each with its own semaphore.
    out_tiles: (cols, engine) per output DMA; recip granularity =
    recip_tiles or in_tile widths (must nest within in_tile boundaries)."""
    import contextlib

    import concourse.bass as bass
    import concourse.mybir as mybir

    f32 = mybir.dt.float32
    in_tiles = [tuple(t) for t in in_tiles]
    out_tiles = [tuple(t) for t in out_tiles]
    assert sum(w for w, _ in in_tiles) == F
    assert sum(w for w, _ in out_tiles) == F
    # only these engines have emit paths below; anything else would be
    # silently dropped (sim then reports a bogus-fast, output-less kernel)
    assert all(e in ("sync", "scalar", "gpsimd") for _, e in in_tiles + out_tiles)
    if recip_tiles is None:
        recip_tiles = [w for w, _ in in_tiles]
    recip_tiles = list(recip_tiles)
    assert sum(recip_tiles) == F

    # Bass.__init__ emits, per engine, a 5-RegisterMove preamble plus 4
    # const-AP memsets and an all-engine startup barrier.  None are used by
    # this kernel and the SP preamble delays the first input DMA by ~300ns,
    # so patch them out for the duration of construction.
    patches = []

    def patch(cls, name, val):
        patches.append((cls, name, name in vars(cls), getattr(cls, name, None)))
        setattr(cls, name, val)

    patch(bass.Bass, "all_engine_barrier", lambda self, **kw: None)
    seen = set()
    for cls_name in dir(bass):
        cls = getattr(bass, cls_name)
        if not isinstance(cls, type) or cls in seen or not cls_name.startswith("Bass"):
            continue
        seen.add(cls)
        if hasattr(cls, "memset"):
            patch(cls, "memset", lambda self, ap, c: None)
        if strip_preamble and hasattr(cls, "preamble"):
            patch(cls, "preamble", lambda self: None)
    try:
        nc = bass.Bass(trn_type="TRN2")
    finally:
        for cls, name, had, orig in patches:
            if had:
                setattr(cls, name, orig)
            else:
                try:
                    delattr(cls, name)
                except AttributeError:
                    pass

    x_d = nc.dram_tensor("x", [P, F], f32, kind="ExternalInput")
    o_d = nc.dram_tensor("out", [P, F], f32, kind="ExternalOutput")

    in_off = [sum(w for w, _ in in_tiles[:i]) for i in range(len(in_tiles))]
    out_off = [sum(w for w, _ in out_tiles[:i]) for i in range(len(out_tiles))]
    r_off = [sum(recip_tiles[:i]) for i in range(len(recip_tiles))]
    n_in = len(in_tiles)
    n_out = len(out_tiles)
    n_r = len(recip_tiles)

    def in_idx_for(a, b):
        """index of the single in-tile containing columns [a, b)."""
        for i in range(n_in):
            if in_off[i] <= a and b <= in_off[i] + in_tiles[i][0]:
                return i
        raise AssertionError(f"recip [{a},{b}) spans in-tile boundary")

    def r_count_for(a, b):
        """number of leading recips that must complete before reading
        columns [a, b) of the result."""
        k = 0
        for i in range(n_r):
            if r_off[i] < b and r_off[i] + recip_tiles[i] > a:
                k = i + 1
        return k

    with contextlib.ExitStack() as st:
        ent = st.enter_context
        xt = ent(nc.sbuf_tensor([P, F], f32))
        ot = ent(nc.sbuf_tensor([P, F], f32))

        s_ins = [ent(nc.semaphore(name=f"s_in{i}")) for i in range(n_in)]
        s_dve = ent(nc.semaphore(name="s_dve"))
        s_od = ent(nc.semaphore(name="s_od"))

        eng_of = {"sync": nc.sync, "scalar": nc.scalar, "vector": nc.vector,
                  "gpsimd": nc.gpsimd}

        def emit_in(i):
            w, e = in_tiles[i]
            c0 = in_off[i]
            eng_of[e].dma_start(
                xt[:, c0:c0 + w], x_d[:, c0:c0 + w]).then_inc(s_ins[i], 16)

        def emit_out(j):
            w, e = out_tiles[j]
            c0 = out_off[j]
            if not attach_waits:
                eng_of[e].wait_ge(s_dve, r_count_for(c0, c0 + w))
            ins = eng_of[e].dma_start(o_d[:, c0:c0 + w], ot[:, c0:c0 + w])
            if attach_waits:
                ins._wait_ge(s_dve, r_count_for(c0, c0 + w))
            # walrus requires every DGE DMA to carry >=1 sem update
            ins.then_inc(s_od, 16)

        # input DMAs go in the entry block, ahead of the branch into blk,
        # so SP's first instruction at t=0 is the first DMACopy
        for i in range(n_in):
            if in_tiles[i][1] == "sync":
                emit_in(i)

        blk = bass.BassBlock(nc, "blk")
        blk.__enter__()

        last_out_eng = out_tiles[-1][1]

        @blk.sync
        def _(sync):
            for j in range(n_out):
                if out_tiles[j][1] == "sync":
                    emit_out(j)
            if clears and last_out_eng == "sync":
                # last waiter of s_dve clears it so the loaded NEFF can be
                # re-executed (runs after this engine's last out-DMA wait,
                # hidden under the DMA flight)
                sync.wait_ge(s_dve, n_r)
                sync.sem_clear(s_dve)
            if final_wait:
                sync.wait_ge(s_od, 16 * n_out)
                sync.sem_clear(s_od)

        @blk.vector
        def _(vector):
            for j in range(n_r):
                w = recip_tiles[j]
                c0 = r_off[j]
                if not attach_waits:
                    vector.wait_ge(s_ins[in_idx_for(c0, c0 + w)], 16)
                ins = nc.vector.reciprocal(ot[:, c0:c0 + w], xt[:, c0:c0 + w])
                if attach_waits:
                    ins._wait_ge(s_ins[in_idx_for(c0, c0 + w)], 16)
                ins.then_inc(s_dve, 1)
            if clears:
                for i in range(n_in):
                    vector.wait_ge(s_ins[i], 16)
                    vector.sem_clear(s_ins[i])

        @blk.scalar
        def _(scalar):
            for i in range(n_in):
                if in_tiles[i][1] == "scalar":
                    emit_in(i)
            for j in range(n_out):
                if out_tiles[j][1] == "scalar":
                    emit_out(j)
            if clears and last_out_eng == "scalar":
                scalar.wait_ge(s_dve, n_r)
                scalar.sem_clear(s_dve)

        if any(e == "gpsimd" for _, e in in_tiles + out_tiles):
            @blk.gpsimd
            def _(gpsimd):
                for i in range(n_in):
                    if in_tiles[i][1] == "gpsimd":
                        emit_in(i)
                for j in range(n_out):
                    if out_tiles[j][1] == "gpsimd":
                        emit_out(j)
                if clears and last_out_eng == "gpsimd":
                    gpsimd.wait_ge(s_dve, n_r)
                    gpsimd.sem_clear(s_dve)

        for engine, last_body in blk.last_body.items():
            with nc.body(
                last_body, parent=nc.cur_bb, allow_existing_parent=True
            ):
                engine.br(blk.end_bb)
        nc.switch_bb(blk.end_bb)
        if end_mode == "drains":
            for eng_type, eng in nc.engines.items():
                d = mybir.InstDrain(
                    name=nc.get_next_instruction_name(),
                    ins=[], outs=[], bass_is_fusable=False,
                )
                d.engine = eng_type
                eng.add_instruction(d)

    return nc


_CACHED = {}

BEST_CONFIG = dict(
    in_tiles=((244, "sync"), (268, "gpsimd")),
    out_tiles=((244, "scalar"), (268, "sync")),
)


def _freeze(v):
    if isinstance(v, (list, tuple)):
        return tuple(_freeze(x) for x in v)
    return v


def _get_nc(**kw):
    key = tuple(sorted((k, _freeze(v)) for k, v in kw.items()))
    if key not in _CACHED:
        _CACHED[key] = _build_bass(**dict(key))
    return _CACHED[key]


def kernel(x: np.ndarray, recip_table_val: np.ndarray = None, **_unused) -> np.ndarray:
    from concourse.bass_utils import run_bass_kernel_spmd

    x = np.ascontiguousarray(np.asarray(x, dtype=np.float32))
    assert x.shape == (N,), x.shape

    nc = _get_nc(**BEST_CONFIG)
    in_maps = [
        {"x": x[i * SHARD:(i + 1) * SHARD].reshape(P, F)} for i in range(N_CORES)
    ]
    res = run_bass_kernel_spmd(nc, in_maps, core_ids=list(range(N_CORES)))
    outs = [res.results[i]["out"].reshape(SHARD) for i in range(N_CORES)]
    return np.concatenate(outs).astype(np.float32)


if __name__ == "__main__":
    rng = np.random.default_rng(0)
    x = (rng.uniform(1.0, 1000.0, N) * np.where(rng.random(N) < 0.5, 1.0, -1.0)).astype(np.float32)
    y = kernel(x)
    print("ok", y[:4], 1.0 / x[:4])


# revision 12
# speedup vs baseline: 1.0130x; 1.0104x over previous
"""Trainium2 Bass kernel for nn_ArithmeticExperts (reciprocal_table).

Reference math per element:
    sign = sign(x); xa = |x|
    exp  = floor(log2(xa)) + 1 ; temp = xa * 2^-exp  (mantissa in [0.5, 1))
    idx  = (temp - 0.5) * 256
    y0   = softmax(-|arange(256) - idx| * 1000) @ table   # sharp softmax
    y    = y0*(2 - temp*y0); y = y*(2 - temp*y)           # 2 Newton steps
    out  = y * 2^-exp * sign

Key observation: the reference's 8-bit table seed + two Newton steps converge
to 1/x at f32 roundoff (seed rel err ~2e-3 -> (2e-3)^4 after two steps, far
below f32 eps), so its output IS 1/x up to a few ulp.  A single DVE
InstReciprocal (IEEE-exact 1/x on TRN2, bitwise-verified by the interpreter
suite and measured 0.0 rel err vs np.reciprocal on these inputs across
repeated device runs) replaces the whole 12-op table+Newton pipeline.

What remains is DMA choreography; per core (65536 elems = [128 part, 512]):
  - THREE input DMAs (184/164/164 cols) on three parallel issue channels,
    all emitted in the entry block so each engine's stream starts at t=0:
    #1 via SP's HWDGE (transfer from t=1300, the floor), #2 via Pool's
    SWDGE (desc-gen on the otherwise-idle Pool engine, transfer from
    ~1749, parallel to HWDGE), #3 as SP's second HWDGE DMA (DGE-ready
    1950; its transfer packs right behind Pool's on the shared DMA
    engines).  Each input has its OWN semaphore: DMA completions across
    queues are not ordered, so a shared counting semaphore would race.
  - three DVE reciprocals in arrival order, each with its input-wait
    ATTACHED to the instruction (waits ride the engine-stage for free;
    standalone EventSemaphore ops cost ~70ns SEQ each).
  - two output HWDGE DMAs: cols 0-183 from ACT as soon as recip0's sem
    fires (its HWDGE gen + transfer complete before the final output needs
    the shared devices), cols 184-511 from SP after the last recip.
    walrus requires every DGE DMA to carry >=1 sem update, so both
    increment s_od (nothing waits on it; outputs are complete when the
    NEFF drains -- verified bit-exact over repeated executions).
  - Bass's per-engine register preamble (5 RegisterMoves/engine, ~300ns
    before SP's first instruction), the const-AP memsets, the all-engine
    startup barrier, and MonotonicSemaphore's Pool reg_mov (61ns ahead of
    Pool's desc-gen) are patched out during Bass construction: this kernel
    uses none of them, and they sit directly on the critical path.

Timeline (cost model): three input transfers land 1562/1982/2215 ->
+907ns DMA sems -> recips 2469-3353 (packed: each starts as its data or
the engine frees) -> out HWDGEs (ACT 2813, SP 3445) -> final transfer
ends 5186 -> +900ns completion-sem propagation = 6086ns, vs 11338ns for
the table+Newton baseline.  Triple-balanced: ACT-out HWDGE-end ~= last
recip's sem (4ns apart), recip1-end ~= recip2's data arrival (2ns), and
the model's remaining terms are hardware constants (2x ~907ns DMA-sem
propagation, 650ns DGE delays, 625-632ns HWDGE desc-gen, bandwidth-bound
transfers, DVE-rate reciprocals).

Dead ends (tried, kept out): SWDGE prepare_only + trigger_dma would cut the
tail to ~4.4us, but this container's walrus rejects InstTriggerDma ("ISA
wrong length", with or without signals/register count); kv_writeback
(normal mode) compiles and prices at 9 descriptors (~51ns transfer) but its
Pool ucode crashes at execution; Pool-issued plain DMAs gain <10ns in the
model and add ucode risk; ACT-engine Reciprocal is rejected by bass for
accuracy and loses to DVE anyway (222-cycle SBUF access); gather/scatter
price at full per-row descriptor cost.

Pure data parallel: 8 cores x 65536 contiguous elements, no collectives.
"""

import sys

if "/opt/trn_rl_repo" not in sys.path:
    sys.path.insert(0, "/opt/trn_rl_repo")

import numpy as np

N = 524288
N_CORES = 8
SHARD = N // N_CORES          # 65536
P = 128
F = SHARD // P                # 512 elements per partition


def _build_bass(in_tiles=((184, "sync"), (164, "gpsimd"), (164, "sync")),
                out_tiles=((184, "scalar"), (328, "sync")),
                recip_tiles=None, strip_preamble=True,
                final_wait=False, end_mode="drains", clears=True,
                attach_waits=True):
    """in_tiles: (cols, engine) per input DMA, each with its own semaphore.
    out_tiles: (cols, engine) per output DMA; recip granularity =
    recip_tiles or in_tile widths (must nest within in_tile boundaries)."""
    import contextlib

    import concourse.bass as bass
    import concourse.mybir as mybir

    f32 = mybir.dt.float32
    in_tiles = [tuple(t) for t in in_tiles]
    out_tiles = [tuple(t) for t in out_tiles]
    assert sum(w for w, _ in in_tiles) == F
    assert sum(w for w, _ in out_tiles) == F
    # only these engines have emit paths below; anything else would be
    # silently dropped (sim then reports a bogus-fast, output-less kernel)
    assert all(e in ("sync", "scalar", "gpsimd") for _, e in in_tiles + out_tiles)
    if recip_tiles is None:
        recip_tiles = [w for w, _ in in_tiles]
    recip_tiles = list(recip_tiles)
    assert sum(recip_tiles) == F

    # Bass.__init__ emits, per engine, a 5-RegisterMove preamble plus 4
    # const-AP memsets and an all-engine startup barrier.  None are used by
    # this kernel and the SP preamble delays the first input DMA by ~300ns,
    # so patch them out for the duration of construction.
    patches = []

    def patch(cls, name, val):
        patches.append((cls, name, name in vars(cls), getattr(cls, name, None)))
        setattr(cls, name, val)

    patch(bass.Bass, "all_engine_barrier", lambda self, **kw: None)

    # MonotonicSemaphore.__init__ emits a reg_mov on Pool at t=0 (61ns ahead
    # of Pool's first DMA desc-gen); we never use monotonic sems, and
    # finalize only needs .sem(), so skip the register setup.
    def _mono_init(self, engine, sem):
        self._engine = engine
        self._sem = sem
        self._reg = None
    patch(bass.MonotonicSemaphore, "__init__", _mono_init)
    seen = set()
    for cls_name in dir(bass):
        cls = getattr(bass, cls_name)
        if not isinstance(cls, type) or cls in seen or not cls_name.startswith("Bass"):
            continue
        seen.add(cls)
        if hasattr(cls, "memset"):
            patch(cls, "memset", lambda self, ap, c: None)
        if strip_preamble and hasattr(cls, "preamble"):
            patch(cls, "preamble", lambda self: None)
    try:
        nc = bass.Bass(trn_type="TRN2")
    finally:
        for cls, name, had, orig in patches:
            if had:
                setattr(cls, name, orig)
            else:
                try:
                    delattr(cls, name)
                except AttributeError:
                    pass

    x_d = nc.dram_tensor("x", [P, F], f32, kind="ExternalInput")
    o_d = nc.dram_tensor("out", [P, F], f32, kind="ExternalOutput")

    in_off = [sum(w for w, _ in in_tiles[:i]) for i in range(len(in_tiles))]
    out_off = [sum(w for w, _ in out_tiles[:i]) for i in range(len(out_tiles))]
    r_off = [sum(recip_tiles[:i]) for i in range(len(recip_tiles))]
    n_in = len(in_tiles)
    n_out = len(out_tiles)
    n_r = len(recip_tiles)

    def in_idx_for(a, b):
        """index of the single in-tile containing columns [a, b)."""
        for i in range(n_in):
            if in_off[i] <= a and b <= in_off[i] + in_tiles[i][0]:
                return i
        raise AssertionError(f"recip [{a},{b}) spans in-tile boundary")

    def r_count_for(a, b):
        """number of leading recips that must complete before reading
        columns [a, b) of the result."""
        k = 0
        for i in range(n_r):
            if r_off[i] < b and r_off[i] + recip_tiles[i] > a:
                k = i + 1
        return k

    with contextlib.ExitStack() as st:
        ent = st.enter_context
        xt = ent(nc.sbuf_tensor([P, F], f32))
        ot = ent(nc.sbuf_tensor([P, F], f32))

        s_ins = [ent(nc.semaphore(name=f"s_in{i}")) for i in range(n_in)]
        s_dve = ent(nc.semaphore(name="s_dve"))
        s_od = ent(nc.semaphore(name="s_od"))

        eng_of = {"sync": nc.sync, "scalar": nc.scalar, "vector": nc.vector,
                  "gpsimd": nc.gpsimd}

        def emit_in(i):
            w, e = in_tiles[i]
            c0 = in_off[i]
            eng_of[e].dma_start(
                xt[:, c0:c0 + w], x_d[:, c0:c0 + w]).then_inc(s_ins[i], 16)

        def emit_out(j):
            w, e = out_tiles[j]
            c0 = out_off[j]
            if not attach_waits:
                eng_of[e].wait_ge(s_dve, r_count_for(c0, c0 + w))
            ins = eng_of[e].dma_start(o_d[:, c0:c0 + w], ot[:, c0:c0 + w])
            if attach_waits:
                ins._wait_ge(s_dve, r_count_for(c0, c0 + w))
            # walrus requires every DGE DMA to carry >=1 sem update
            ins.then_inc(s_od, 16)

        # input DMAs go in the entry block, ahead of the branch into blk:
        # each engine's entry stream starts at t=0, so SP's first DMACopy
        # issues immediately and Pool's SWDGE desc-gen / ACT's HWDGE slot
        # start as early as possible
        for i in range(n_in):
            emit_in(i)

        blk = bass.BassBlock(nc, "blk")
        blk.__enter__()

        last_out_eng = out_tiles[-1][1]

        @blk.sync
        def _(sync):
            for j in range(n_out):
                if out_tiles[j][1] == "sync":
                    emit_out(j)
            if clears and last_out_eng == "sync":
                # last waiter of s_dve clears it so the loaded NEFF can be
                # re-executed (runs after this engine's last out-DMA wait,
                # hidden under the DMA flight)
                sync.wait_ge(s_dve, n_r)
                sync.sem_clear(s_dve)
            if final_wait:
                sync.wait_ge(s_od, 16 * n_out)
                sync.sem_clear(s_od)

        @blk.vector
        def _(vector):
            for j in range(n_r):
                w = recip_tiles[j]
                c0 = r_off[j]
                if not attach_waits:
                    vector.wait_ge(s_ins[in_idx_for(c0, c0 + w)], 16)
                ins = nc.vector.reciprocal(ot[:, c0:c0 + w], xt[:, c0:c0 + w])
                if attach_waits:
                    ins._wait_ge(s_ins[in_idx_for(c0, c0 + w)], 16)
                ins.then_inc(s_dve, 1)
            if clears:
                for i in range(n_in):
                    vector.wait_ge(s_ins[i], 16)
                    vector.sem_clear(s_ins[i])

        @blk.scalar
        def _(scalar):
            for j in range(n_out):
                if out_tiles[j][1] == "scalar":
                    emit_out(j)
            if clears and last_out_eng == "scalar":
                scalar.wait_ge(s_dve, n_r)
                scalar.sem_clear(s_dve)

        if any(e == "gpsimd" for _, e in in_tiles + out_tiles):
            @blk.gpsimd
            def _(gpsimd):
                for j in range(n_out):
                    if out_tiles[j][1] == "gpsimd":
                        emit_out(j)
                if clears and last_out_eng == "gpsimd":
                    gpsimd.wait_ge(s_dve, n_r)
                    gpsimd.sem_clear(s_dve)

        for engine, last_body in blk.last_body.items():
            with nc.body(
                last_body, parent=nc.cur_bb, allow_existing_parent=True
            ):
                engine.br(blk.end_bb)
        nc.switch_bb(blk.end_bb)
        if end_mode == "drains":
            for eng_type, eng in nc.engines.items():
                d = mybir.InstDrain(
                    name=nc.get_next_instruction_name(),
                    ins=[], outs=[], bass_is_fusable=False,
                )
                d.engine = eng_type
                eng.add_instruction(d)

    return nc


_CACHED = {}

BEST_CONFIG = dict(
    in_tiles=((184, "sync"), (164, "gpsimd"), (164, "sync")),
    out_tiles=((184, "scalar"), (328, "sync")),
)


def _freeze(v):
    if isinstance(v, (list, tuple)):
        return tuple(_freeze(x) for x in v)
    return v


def _get_nc(**kw):
    key = tuple(sorted((k, _freeze(v)) for k, v in kw.items()))
    if key not in _CACHED:
        _CACHED[key] = _build_bass(**dict(key))
    return _CACHED[key]


def kernel(x: np.ndarray, recip_table_val: np.ndarray = None, **_unused) -> np.ndarray:
    from concourse.bass_utils import run_bass_kernel_spmd

    x = np.ascontiguousarray(np.asarray(x, dtype=np.float32))
    assert x.shape == (N,), x.shape

    nc = _get_nc(**BEST_CONFIG)
    in_maps = [
        {"x": x[i * SHARD:(i + 1) * SHARD].reshape(P, F)} for i in range(N_CORES)
    ]
    res = run_bass_kernel_spmd(nc, in_maps, core_ids=list(range(N_CORES)))
    outs = [res.results[i]["out"].reshape(SHARD) for i in range(N_CORES)]
    return np.concatenate(outs).astype(np.float32)


if __name__ == "__main__":
    rng = np.random.default_rng(0)
    x = (rng.uniform(1.0, 1000.0, N) * np.where(rng.random(N) < 0.5, 1.0, -1.0)).astype(np.float32)
    y = kernel(x)
    print("ok", y[:4], 1.0 / x[:4])
